# revision 7
# baseline (speedup 1.0000x reference)
"""AnomalyTransformer forward on 8 TRN2 NeuronCores.

Sharding: collective-free. Core c computes batch element b=c//2 end-to-end
(pairs duplicate the per-token pipeline), but each core materializes/writes
only 4 of the 8 attention heads' series/prior tensors. Odd cores see the
head-blocks of Wq/Wk/Wv/Wsig (cols) and Wo (rows) pre-permuted host-side so
the SPMD graph always outputs "heads 0-3" of its own view.

All matmuls run fp32 via float32r (full-rate at moving-dim>=256). Softmax
skips max-subtraction (|scores| <~ 1 by construction). The softmax recip is
folded in per-partition after exp (series output) and after the unnormalized
attn^T matmul (attention path), so no LxL transpose of series is ever done:
exp(scores^T) is computed by a second K=64 matmul with q/k swapped.
"""
import sys

sys.path.insert(0, '/opt/trn_rl_repo')

import math
import numpy as np

B, L, CIN, D, H, NL, DFF = 4, 512, 38, 512, 8, 3, 2048
E = D // H           # 64
HH = H // 2          # 4 output heads per core
NC = 4               # 128-token chunks per L
ND = 4               # 128-row chunks per D
NK = DFF // 128      # 16 dff chunks
P = 128
LN3 = math.log(3.0)
INV_SQRT_2PI = 1.0 / math.sqrt(2.0 * math.pi)

_CACHE = {}


# ---------------------------------------------------------------------------
# walrus on this image rejects instructions with more than one sync wait.
# Split excess waits onto nofuse nops on the same engine.
# ---------------------------------------------------------------------------

def _patch_drain_wait_split():
    from concourse import tile as tile_mod
    import bass_rust

    if getattr(tile_mod.TileContext._drain_and_barrier, '_waitfix', False):
        return

    def patched(self, tick_clock, wait_clock):
        from concourse.vector_clock import ScopedClock

        nc = self.nc
        drain_inst = nc.sync.drain()
        wait_clock.add_sem_waits(
            drain_inst.ins, ScopedClock({None: tick_clock.global_clock})
        )
        si = drain_inst.ins.sync_info
        waits = list(si.on_wait)
        if len(waits) > 1:
            si.on_wait = waits[:1]
            drain_inst.ins.sync_info = si
            for w in waits[1:]:
                nop = nc.sync.nop(nofuse=True, hint="drain_wait_split")
                nop.ins.sync_info = bass_rust.SyncInfo(on_wait=[w], on_update=[])

        nc.all_engine_barrier()
        assert self.sems is not None
        popped = nc._tile_sem_poison_stack.pop()
        assert popped is self._sem_poison
        nc.clear_and_free_semaphores(list(self.sems.allocated().values()))
        nc.all_engine_barrier()

    patched._waitfix = True
    tile_mod.TileContext._drain_and_barrier = patched


def _fix_multi_waits(nc):
    """Post-pass: any instruction carrying >1 sem waits gets the extra waits
    hoisted onto nofuse nops inserted just before it (same engine)."""
    from concourse import mybir

    n = 0
    for f in nc.m.functions:
        for bb in f.blocks:
            insts = list(bb.instructions)
            out = []
            changed = False
            for ins in insts:
                si = ins.sync_info
                if si is not None and len(si.on_wait) > 1:
                    waits = list(si.on_wait)
                    for w in waits[:-1]:
                        n += 1
                        nop = mybir.InstNoOp(
                            name=f"waitfix-{n}",
                            engine=ins.engine,
                            bass_nofuse=True,
                            sync_info=mybir.SyncInfo(on_wait=[w], on_update=[]),
                        )
                        out.append(nop)
                    si.on_wait = waits[-1:]
                    ins.sync_info = si
                    changed = True
                out.append(ins)
            if changed:
                bb.instructions = out
    return n


# ---------------------------------------------------------------------------
# Device graph
# ---------------------------------------------------------------------------

def _build_nc():
    import concourse.bass as bass
    import concourse.mybir as mybir
    from concourse.tile import TileContext
    from concourse.masks import make_identity

    f32 = mybir.dt.float32
    f32r = mybir.dt.float32r
    nc = bass.Bass()

    def param(name, shape, dt=None):
        return nc.declare_dram_parameter(
            name, list(shape), dt or f32, isOutput=False
        )

    xcatT_d = param("xcatT", (3 * CIN, L), f32r)
    tokw_d = param("tokw", (3 * CIN, D), f32r)
    pos_d = param("pos", (L, D))
    dist2_d = param("dist2", (L, L))
    wq_d = param("wq", (NL, D, D), f32r)
    wk_d = param("wk", (NL, D, D), f32r)
    wv_d = param("wv", (NL, D, D), f32r)
    bqk_d = param("bqk", (P, NL, 2, ND))      # [p, l, (q|k), hp]
    wsig_d = param("wsig", (NL, D, HH), f32r)
    bsigb_d = param("bsigb", (P, NL, HH))     # broadcast
    wo_d = param("wo", (NL, D, D), f32r)
    conv1_d = param("conv1", (NL, D, DFF), f32r)
    c1b_d = param("c1b", (P, NL, NK))         # [p, l, kd]
    conv2_d = param("conv2", (NL, DFF, D), f32r)
    bc512_d = param("bc512", (NL, 7, P, D))   # bv,bo,c2b,ln1g,ln1b,ln2g,ln2b
    lnf_d = param("lnf", (2, P, D))
    projw_d = param("projw", (D, CIN), f32r)
    projb_d = param("projb", (P, CIN))

    series_o = nc.declare_dram_parameter("series_o", [NL, HH, L, L], f32, isOutput=True)
    prior_o = nc.declare_dram_parameter("prior_o", [NL, HH, L, L], f32, isOutput=True)
    out_o = nc.declare_dram_parameter("out_o", [L, CIN], f32, isOutput=True)

    AluOp = mybir.AluOpType
    Act = mybir.ActivationFunctionType

    # register float constants used as activation biases
    for cval in (LN3 * 1e-5, 1e-5):
        t = nc.alloc_sbuf_tensor(f"const-f32-{cval}", [128, 1], f32)
        nc.gpsimd.memset(t.ap(), cval)
        nc.const_aps.aps[(f32, cval)] = t.ap()
    nc.all_engine_barrier()

    def r(ap):
        return ap

    with TileContext(nc) as tc:
        with (
            tc.tile_pool(name="const", bufs=1) as constp,
            tc.tile_pool(name="acts", bufs=1) as actp,
            tc.tile_pool(name="wstream", bufs=1) as wsp,
            tc.tile_pool(name="small", bufs=2) as smp,
            tc.tile_pool(name="heads", bufs=4) as hdp,
            tc.tile_pool(name="dram", bufs=4, space="DRAM") as drp,
        ):
            # ---- constants ----
            ident = constp.tile([P, P], f32, name="ident")
            make_identity(nc, ident)
            dist2_sb = constp.tile([P, NC, L], f32, name="dist2_sb")
            nc.sync.dma_start(dist2_sb[:], dist2_d.rearrange("(c p) s -> p c s", p=P))
            bqk_sb = constp.tile([P, NL, 2, ND], f32, name="bqk_sb")
            nc.sync.dma_start(bqk_sb[:], bqk_d[:])
            c1b_sb = constp.tile([P, NL, NK], f32, name="c1b_sb")
            nc.sync.dma_start(c1b_sb[:], c1b_d[:])
            bsig_sb = constp.tile([P, NL, HH], f32, name="bsig_sb")
            nc.sync.dma_start(bsig_sb[:], bsigb_d[:])

            # ---- persistent activation tiles (rotated per layer via tags) ----
            def new_enc():
                return actp.tile([P, NC, D], f32, tag="enc", name="enc")

            def new_encT():
                return actp.tile([P, ND, L], f32r, tag="encT", name="encT")

            def transpose_512(dst, src, tpool):
                # dst[p, k, l-chunk] = src[l, c, k-block].T
                for k in range(ND):
                    tp = tpool.tile([P, L], f32, tag="tp", bufs=2, name="tp")
                    for c in range(NC):
                        nc.tensor.transpose(
                            tp[:, c * P:(c + 1) * P],
                            src[:, c, k * P:(k + 1) * P],
                            ident[:],
                        )
                    if k % 2 == 0:
                        nc.vector.tensor_copy(dst[:, k, :], tp[:])
                    else:
                        nc.scalar.activation(dst[:, k, :], tp[:], Act.Identity)

            # ---- embedding ----
            with tc.tile_pool(name="embed", bufs=1) as ep:
                xcatT_sb = ep.tile([3 * CIN, L], f32r, name="xcatT_sb")
                nc.sync.dma_start(xcatT_sb[:], xcatT_d[:])
                tokw_sb = ep.tile([3 * CIN, D], f32r, name="tokw_sb")
                nc.sync.dma_start(tokw_sb[:], tokw_d[:])
                pos_sb = ep.tile([P, NC, D], f32, name="pos_sb")
                nc.sync.dma_start(pos_sb[:], pos_d.rearrange("(c p) d -> p c d", p=P))

                enc = new_enc()
                with tc.tile_pool(name="pemb", bufs=1, space="PSUM") as pe_p:
                    for c in range(NC):
                        emb = pe_p.tile([P, D], f32, tag="mm1", bufs=2, name="emb")
                        nc.tensor.matmul(
                            emb[:], r(xcatT_sb[:, c * P:(c + 1) * P]), r(tokw_sb[:]),
                            start=True, stop=True,
                        )
                        nc.vector.tensor_tensor(
                            enc[:, c, :], emb[:], pos_sb[:, c, :], AluOp.add
                        )
            encT = new_encT()
            with tc.tile_pool(name="ptr0", bufs=1, space="PSUM") as tp_p:
                transpose_512(encT, enc, tp_p)

            # ---- layers ----
            for l in range(NL):
                # --- stream this layer's weights ---
                wv_t = wsp.tile([P, ND, D], f32r, tag="wv", name="wv_t")
                nc.sync.dma_start(wv_t[:], wv_d[l].rearrange("(k p) n -> p k n", p=P))
                wq_t = []
                wk_t = []
                for hp in range(ND):
                    t = wsp.tile([P, ND, P], f32r, tag=f"wq{hp}", name=f"wq_t{hp}")
                    nc.sync.dma_start(
                        t[:],
                        wq_d[l, :, hp * P:(hp + 1) * P].rearrange("(k p) m -> p k m", p=P),
                    )
                    wq_t.append(t)
                    t = wsp.tile([P, ND, P], f32r, tag=f"wk{hp}", name=f"wk_t{hp}")
                    nc.sync.dma_start(
                        t[:],
                        wk_d[l, :, hp * P:(hp + 1) * P].rearrange("(k p) m -> p k m", p=P),
                    )
                    wk_t.append(t)
                wo_t = []
                for hp in range(ND):
                    t = wsp.tile([P, D], f32r, tag=f"wo{hp}", name=f"wo_t{hp}")
                    nc.sync.dma_start(t[:], wo_d[l, hp * P:(hp + 1) * P, :])
                    wo_t.append(t)
                wsig_t = wsp.tile([P, ND, HH], f32r, tag="wsig", name="wsig_t")
                nc.sync.dma_start(wsig_t[:], wsig_d[l].rearrange("(k p) h -> p k h", p=P))
                bc = wsp.tile([P, 7, D], f32, tag="bc", name="bc")
                nc.sync.dma_start(bc[:], bc512_d[l].rearrange("i p d -> p i d"))
                bv_b, bo_b, c2b_b = bc[:, 0, :], bc[:, 1, :], bc[:, 2, :]
                ln1g_b, ln1b_b = bc[:, 3, :], bc[:, 4, :]
                ln2g_b, ln2b_b = bc[:, 5, :], bc[:, 6, :]

                # --- v (natural: [p, s-chunk, he]) ---
                pA = tc.alloc_tile_pool(name="pA", bufs=1, space="PSUM")
                v_sb = actp.tile([P, NC, D], f32r, tag="v", name="v_sb")
                for sc in range(NC):
                    vp = pA.tile([P, D], f32, tag="mm1", bufs=2, name="vp")
                    for k in range(ND):
                        nc.tensor.matmul(
                            vp[:], r(encT[:, k, sc * P:(sc + 1) * P]), r(wv_t[:, k, :]),
                            start=(k == 0), stop=(k == ND - 1),
                        )
                    nc.vector.tensor_tensor(v_sb[:, sc, :], vp[:], bv_b, AluOp.add)

                # --- sig -> sigma -> a, c (output heads only) ---
                siga = smp.tile([P, NC, HH], f32, tag="siga", name="siga")
                sigc = smp.tile([P, NC, HH], f32, tag="sigc", name="sigc")
                for c in range(NC):
                    sp = pA.tile([P, HH], f32, tag="sig", bufs=2, name="sp")
                    for k in range(ND):
                        nc.tensor.matmul(
                            sp[:], r(encT[:, k, c * P:(c + 1) * P]), r(wsig_t[:, k, :]),
                            start=(k == 0), stop=(k == ND - 1),
                        )
                    sg = smp.tile([P, HH], f32, tag="sg", bufs=3, name="sg")
                    nc.vector.tensor_tensor(sg[:], sp[:], bsig_sb[:, l, :], AluOp.add)
                    # sigma = 3**(sigmoid(5*sig)+1e-5) - 1
                    nc.scalar.activation(sg[:], sg[:], Act.Sigmoid, scale=5.0)
                    nc.scalar.activation(sg[:], sg[:], Act.Exp, scale=LN3, bias=LN3 * 1e-5)
                    nc.vector.tensor_scalar(
                        sg[:], sg[:], 1.0, None, AluOp.subtract
                    )  # sigma
                    s2 = smp.tile([P, HH], f32, tag="s2", bufs=3, name="s2")
                    nc.vector.tensor_tensor(s2[:], sg[:], sg[:], AluOp.mult)
                    nc.vector.reciprocal(s2[:], s2[:])
                    nc.vector.tensor_scalar(
                        siga[:, c, :], s2[:], -0.5, None, AluOp.mult
                    )  # a = -1/(2 sigma^2)
                    nc.vector.reciprocal(sg[:], sg[:])
                    nc.vector.tensor_scalar(
                        sigc[:, c, :], sg[:], INV_SQRT_2PI, None, AluOp.mult
                    )  # c = 1/(sqrt(2pi) sigma)

                pA.release()

                # --- prior (output heads) ---
                for h in range(HH):
                    for c in range(NC):
                        pr = hdp.tile([P, L], f32, tag="prior", bufs=3, name="pr")
                        nc.scalar.activation(
                            pr[:], dist2_sb[:, c, :], Act.Exp,
                            scale=siga[:, c, h:h + 1],
                        )
                        nc.vector.tensor_scalar_mul(pr[:], pr[:], sigc[:, c, h:h + 1])
                        nc.sync.dma_start(prior_o[l, h, c * P:(c + 1) * P, :], pr[:])

                # --- attention ---
                pB = tc.alloc_tile_pool(name="pB", bufs=1, space="PSUM")
                attnT2 = []
                for hp in range(ND):
                    qkt = []
                    for j, wt in ((0, wq_t[hp]), (1, wk_t[hp])):
                        qp = pB.tile([P, L], f32, tag="qkt", bufs=2, name="qp")
                        for k in range(ND):
                            nc.tensor.matmul(
                                qp[:], r(wt[:, k, :]), r(encT[:, k, :]),
                                start=(k == 0), stop=(k == ND - 1),
                            )
                        qs = smp.tile([P, L], f32r, tag=f"qk{j}", bufs=2, name="qs")
                        nc.vector.tensor_scalar_add(
                            qs[:], qp[:], bqk_sb[:, l, j, hp:hp + 1]
                        )
                        qkt.append(qs)
                    qT2, kT2 = qkt

                    rb = smp.tile([P, L], f32, tag="recipb", bufs=2, name="rb")
                    a2 = hdp.tile([P, L], f32r, tag="attnT2", name="a2")
                    for h2 in range(2):
                        hh = hp * 2 + h2
                        base = h2 * 64
                        qT = qT2[base:base + 64, :]
                        kT = kT2[base:base + 64, :]
                        sums = smp.tile([P, NC], f32, tag="sums", bufs=4, name="sums")
                        exps = []
                        for c in range(NC):
                            scp = pB.tile([P, L], f32, tag="sc", bufs=2, name="scp")
                            nc.tensor.matmul(
                                scp[:], r(qT[:, c * P:(c + 1) * P]), r(kT),
                                start=True, stop=True,
                            )
                            ex = hdp.tile([P, L], f32, tag="exp", name="ex")
                            nc.scalar.activation(
                                ex[:], scp[:], Act.Exp, scale=0.125,
                                accum_out=sums[:, c:c + 1],
                            )
                            exps.append(ex)
                        recip = smp.tile([P, NC], f32, tag="recip", bufs=4, name="recip")
                        nc.vector.reciprocal(recip[:], sums[:])
                        if hp < 2:
                            for c in range(NC):
                                nc.vector.tensor_scalar_mul(
                                    exps[c][:], exps[c][:], recip[:, c:c + 1]
                                )
                                nc.sync.dma_start(
                                    series_o[l, hh, c * P:(c + 1) * P, :], exps[c][:]
                                )
                        # recip -> DRAM (l-order) -> broadcast rows [base:base+64]
                        rd = drp.tile([L], f32, tag="rd", name="rd")
                        nc.sync.dma_start(
                            rd.rearrange("(c p) -> p c", p=P), recip[:]
                        )
                        nc.sync.dma_start(
                            rb[base:base + 64, :],
                            rd[:].unsqueeze(0).partition_broadcast(64),
                        )
                        # scores^T -> exp -> attnT accumulation
                        ap = pB.tile([64, L], f32, tag="attnT", bufs=2, name="ap")
                        for sc in range(NC):
                            stp = pB.tile([P, L], f32, tag="scT", bufs=2, name="stp")
                            nc.tensor.matmul(
                                stp[:], r(kT[:, sc * P:(sc + 1) * P]), r(qT),
                                start=True, stop=True,
                            )
                            exT = hdp.tile([P, L], f32r, tag="expT", name="exT")
                            nc.scalar.activation(exT[:], stp[:], Act.Exp, scale=0.125)
                            nc.tensor.matmul(
                                ap[:],
                                r(v_sb[:, sc, hh * 64:(hh + 1) * 64]),
                                r(exT[:]),
                                start=(sc == 0), stop=(sc == NC - 1),
                            )
                        nc.vector.tensor_tensor(
                            a2[base:base + 64, :], ap[:], rb[base:base + 64, :],
                            AluOp.mult,
                        )
                    attnT2.append(a2)

                pB.release()

                # --- Wo + residual + LN1 -> x1 ---
                pC = tc.alloc_tile_pool(name="pC", bufs=1, space="PSUM")
                x1 = actp.tile([P, NC, D], f32, tag="x1", name="x1")
                zres = actp.tile([P, NC, D], f32, tag="v", name="zres")
                for c in range(NC):
                    wp = pC.tile([P, D], f32, tag="mm1", bufs=2, name="wp")
                    for hp in range(ND):
                        nc.tensor.matmul(
                            wp[:], r(attnT2[hp][:, c * P:(c + 1) * P]), r(wo_t[hp][:]),
                            start=(hp == 0), stop=(hp == ND - 1),
                        )
                    z = zres[:, c, :]
                    nc.vector.tensor_tensor(z, wp[:], enc[:, c, :], AluOp.add)
                    nc.vector.tensor_tensor(z, z, bo_b, AluOp.add)
                    _layernorm(nc, tc, smp, x1[:, c, :], z, ln1g_b, ln1b_b, Act, AluOp)

                x1T = actp.tile([P, ND, L], f32r, tag="x1T", name="x1T")
                transpose_512(x1T, x1, pC)
                pC.release()

                # --- FFN ---
                pD = tc.alloc_tile_pool(name="pD", bufs=1, space="PSUM")
                ency = pD.tile([P, NC, D], f32, tag="ency", name="ency")
                for kd in range(NK):
                    c1t = wsp.tile([P, ND, P], f32r, tag="c1", bufs=3, name="c1t")
                    nc.sync.dma_start(
                        c1t[:],
                        conv1_d[l, :, kd * P:(kd + 1) * P].rearrange(
                            "(k p) m -> p k m", p=P
                        ),
                    )
                    c2t = wsp.tile([P, D], f32r, tag="c2", bufs=3, name="c2t")
                    nc.sync.dma_start(c2t[:], conv2_d[l, kd * P:(kd + 1) * P, :])
                    yp = pD.tile([P, L], f32, tag="yT", bufs=2, name="yp")
                    for k in range(ND):
                        nc.tensor.matmul(
                            yp[:], r(c1t[:, k, :]), r(x1T[:, k, :]),
                            start=(k == 0), stop=(k == ND - 1),
                        )
                    gt = wsp.tile([P, L], f32r, tag="gt", bufs=3, name="gt")
                    nc.scalar.activation(
                        gt[:], yp[:], Act.Gelu, bias=c1b_sb[:, l, kd:kd + 1]
                    )
                    for c in range(NC):
                        nc.tensor.matmul(
                            ency[:, c, :], r(gt[:, c * P:(c + 1) * P]), r(c2t[:]),
                            start=(kd == 0), stop=(kd == NK - 1),
                        )

                # --- residual + LN2 -> enc (next layer) ---
                enc = new_enc()
                for c in range(NC):
                    z = zres[:, c, :]
                    nc.vector.tensor_tensor(z, ency[:, c, :], x1[:, c, :], AluOp.add)
                    nc.vector.tensor_tensor(z, z, c2b_b, AluOp.add)
                    _layernorm(nc, tc, smp, enc[:, c, :], z, ln2g_b, ln2b_b, Act, AluOp)
                encT = new_encT()
                transpose_512(encT, enc, pD)
                pD.release()

            # ---- final LN + projection ----
            lnf_sb = constp.tile([P, 2, D], f32, name="lnf_sb")
            nc.sync.dma_start(lnf_sb[:], lnf_d.rearrange("i p d -> p i d"))
            projw_t = constp.tile([P, ND, CIN], f32r, name="projw_t")
            nc.sync.dma_start(projw_t[:], projw_d.rearrange("(k p) n -> p k n", p=P))
            projb_sb = constp.tile([P, CIN], f32, name="projb_sb")
            nc.sync.dma_start(projb_sb[:], projb_d[:])

            encf = actp.tile([P, NC, D], f32, tag="x1", name="encf")
            for c in range(NC):
                _layernorm(
                    nc, tc, smp, encf[:, c, :], enc[:, c, :],
                    lnf_sb[:, 0, :], lnf_sb[:, 1, :], Act, AluOp,
                )
            pF = tc.alloc_tile_pool(name="pF", bufs=1, space="PSUM")
            encfT = actp.tile([P, ND, L], f32r, tag="x1T", name="encfT")
            transpose_512(encfT, encf, pF)
            for c in range(NC):
                op = pF.tile([P, CIN], f32, tag="outp", bufs=2, name="op")
                for k in range(ND):
                    nc.tensor.matmul(
                        op[:], r(encfT[:, k, c * P:(c + 1) * P]), r(projw_t[:, k, :]),
                        start=(k == 0), stop=(k == ND - 1),
                    )
                ot = smp.tile([P, CIN], f32, tag="ot", bufs=2, name="ot")
                nc.vector.tensor_tensor(ot[:], op[:], projb_sb[:], AluOp.add)
                nc.sync.dma_start(out_o[c * P:(c + 1) * P, :], ot[:])
            pF.release()

    _fix_multi_waits(nc)
    return nc


def _layernorm(nc, tc, smp, out_ap, z_ap, g_b, b_b, Act, AluOp):
    """out = (z - mean(z)) * rsqrt(var(z)+eps) * g + b over free dim (512)."""
    import concourse.mybir as mybir

    st6 = smp.tile([P, 6], mybir.dt.float32, tag="st6", bufs=2, name="st6")
    nc.vector.bn_stats(st6[:], z_ap)
    mv = smp.tile([P, 2], mybir.dt.float32, tag="mv", bufs=2, name="mv")
    nc.vector.bn_aggr(mv[:], st6[:])
    rs = smp.tile([P, 1], mybir.dt.float32, tag="rs", bufs=2, name="rs")
    nc.scalar.activation(rs[:], mv[:, 1:2], Act.Sqrt, bias=1e-5)
    nc.vector.reciprocal(rs[:], rs[:])
    nc.vector.tensor_scalar(
        out_ap, z_ap, mv[:, 0:1], rs[:], AluOp.subtract, AluOp.mult
    )
    nc.vector.tensor_tensor(out_ap, out_ap, g_b, AluOp.mult)
    nc.vector.tensor_tensor(out_ap, out_ap, b_b, AluOp.add)


# ---------------------------------------------------------------------------
# Host-side staging
# ---------------------------------------------------------------------------

def _pos_embedding():
    pos = np.arange(L, dtype=np.float32)[:, None]
    div = np.exp(np.arange(0, D, 2, dtype=np.float32) * -(math.log(10000.0) / D))
    pe = np.zeros((L, D), dtype=np.float32)
    pe[:, 0::2] = np.sin(pos * div)
    pe[:, 1::2] = np.cos(pos * div)
    return pe


def _bcast(v):
    return np.broadcast_to(np.asarray(v, np.float32)[None, :], (P, v.shape[-1])).copy()


def _stage_core(x, p, b, half):
    xb = np.asarray(x[b], dtype=np.float32)
    xcat = np.concatenate(
        [np.roll(xb, 1, axis=0), xb, np.roll(xb, -1, axis=0)], axis=1
    )

    def permc(w):
        w = np.asarray(w, np.float32)
        wh = w.reshape(w.shape[:-1] + (H, w.shape[-1] // H))
        if half:
            wh = np.concatenate([wh[..., HH:, :], wh[..., :HH, :]], axis=-2)
        return np.ascontiguousarray(wh.reshape(w.shape))

    def permr(w):
        w = np.asarray(w, np.float32)
        wh = w.reshape(w.shape[:-2] + (H, w.shape[-2] // H, w.shape[-1]))
        if half:
            wh = np.concatenate([wh[..., HH:, :, :], wh[..., :HH, :, :]], axis=-3)
        return np.ascontiguousarray(wh.reshape(w.shape))

    wsig = np.asarray(p['Wsig'], np.float32)
    bsig = np.asarray(p['bsig'], np.float32)
    if half:
        wsig = wsig[:, :, HH:]
        bsig = bsig[:, HH:]
    else:
        wsig = wsig[:, :, :HH]
        bsig = bsig[:, :HH]

    bq = permc(p['bq'])  # (NL, D)
    bk = permc(p['bk'])
    # bqk: [p, l, j, hp] with he = hp*128 + p
    bqk = np.zeros((P, NL, 2, ND), np.float32)
    for l in range(NL):
        for hp in range(ND):
            bqk[:, l, 0, hp] = bq[l, hp * P:(hp + 1) * P]
            bqk[:, l, 1, hp] = bk[l, hp * P:(hp + 1) * P]

    c1b = np.asarray(p['conv1_b'], np.float32)  # (NL, DFF)
    c1bt = np.zeros((P, NL, NK), np.float32)
    for l in range(NL):
        c1bt[:, l, :] = c1b[l].reshape(NK, P).T

    bc512 = np.zeros((NL, 7, P, D), np.float32)
    bv = permc(p['bv'])
    for l in range(NL):
        for i, vec in enumerate([
            bv[l], np.asarray(p['bo'], np.float32)[l],
            np.asarray(p['conv2_b'], np.float32)[l],
            np.asarray(p['ln1_g'], np.float32)[l], np.asarray(p['ln1_b'], np.float32)[l],
            np.asarray(p['ln2_g'], np.float32)[l], np.asarray(p['ln2_b'], np.float32)[l],
        ]):
            bc512[l, i] = np.broadcast_to(vec[None, :], (P, D))

    lnf = np.stack([
        _bcast(np.asarray(p['lnf_g'], np.float32)),
        _bcast(np.asarray(p['lnf_b'], np.float32)),
    ])

    bsigb = np.zeros((P, NL, HH), np.float32)
    for l in range(NL):
        bsigb[:, l, :] = np.broadcast_to(bsig[l][None, :], (P, HH))

    idx = np.arange(L, dtype=np.float32)
    return {
        'xcatT': np.ascontiguousarray(xcat.T),
        'tokw': np.ascontiguousarray(np.asarray(p['tok_w'], np.float32).reshape(3 * CIN, D)),
        'pos': _pos_embedding(),
        'dist2': (idx[:, None] - idx[None, :]) ** 2,
        'wq': permc(p['Wq']), 'wk': permc(p['Wk']), 'wv': permc(p['Wv']),
        'bqk': bqk,
        'wsig': np.ascontiguousarray(wsig), 'bsigb': bsigb,
        'wo': permr(p['Wo']),
        'conv1': np.asarray(p['conv1_w'], np.float32),
        'c1b': c1bt,
        'conv2': np.asarray(p['conv2_w'], np.float32),
        'bc512': bc512,
        'lnf': lnf,
        'projw': np.asarray(p['proj_w'], np.float32),
        'projb': _bcast(np.asarray(p['proj_b'], np.float32)),
    }


def kernel(x=None, params=None, **kw):
    from concourse.bass_utils import run_bass_kernel_spmd

    _patch_drain_wait_split()
    if 'nc' not in _CACHE:
        _CACHE['nc'] = _build_nc()
    nc = _CACHE['nc']

    x = np.asarray(x, np.float32)
    p = {k: np.asarray(v) for k, v in params.items()}

    in_maps = []
    for core in range(8):
        in_maps.append(_stage_core(x, p, core // 2, core % 2))

    res = run_bass_kernel_spmd(nc, in_maps, list(range(8)))
    results = res.results

    out = np.zeros((B, L, CIN), np.float32)
    series = np.zeros((NL, B, H, L, L), np.float32)
    prior = np.zeros((NL, B, H, L, L), np.float32)
    for core in range(8):
        b, half = core // 2, core % 2
        rr = results[core]
        if half == 0:
            out[b] = rr["out_o"]
        series[:, b, half * HH:(half + 1) * HH] = rr["series_o"]
        prior[:, b, half * HH:(half + 1) * HH] = rr["prior_o"]
    return out, series, prior
